# revision 23
# baseline (speedup 1.0000x reference)
"""CosineAttention on 8 TRN2 NeuronCores — v3.

Sharding: head-parallel attention + split AllToAll shard-transpose +
token-parallel out-projection (one head per core, both batches).

Key structure (per core):
  stage 1 (per 512-token chunk): [q;k]T and vT via weight-stationary bf16
    matmuls over resident xT; vT XBAR-transposed (Sync queue) into packed
    [j, 64] vo tiles with a trailing ones column; all sqrt/rsqrt math runs
    as Ln -> Exp(scale) on ACT so ONE activation table serves the whole
    kernel (zero reloads).
  phase 2 (per 1024-token i-chunk): simT = kraw^T qn per j-tile into a
    2-bank PSUM pair, ONE exp instruction per j-tile ([128, 2x512], scale
    AP = 1/|k|), attn@[v|1] accumulates a single PSUM group; softmax
    denominator reciprocal via reciprocal_approx_fast (DVE) + a
    contraction-1 PE matmul broadcast (keeps the Pool queue free for the
    collectives).
  stage-1 work for later chunks is interleaved into phase 2 at j-tile
    granularity so neither ACT nor PE ever drains.
  TWO AllToAlls (one per batch, 256-token blocks): the first overlaps
    batch-1 attention; out-projection of batch 0 overlaps batch-1 tail.
"""

import collections

import numpy as np
import ml_dtypes

import concourse.bass as bass
import concourse.tile as tile
from concourse import bacc
import concourse.mybir as mybir
from concourse import bass_utils

f32 = mybir.dt.float32
bf16 = mybir.dt.bfloat16
AF = mybir.ActivationFunctionType
ALU = mybir.AluOpType

N_CORES = 8
HEADS = 8
D = 64            # head dim
B = 2             # batch
SEQ = 2048        # tokens per batch
DIM = 512         # model dim
NTOK = B * SEQ    # 4096

S1C = 512         # stage-1 token chunk
NS1 = NTOK // S1C          # 8
JPC = S1C // 128           # 4 j-tiles per stage-1 chunk
JPB = SEQ // 128           # 16 j-tiles per batch
P2C = 1024        # phase-2 i-chunk
HW = P2C // 2
BLK = 256         # AllToAll token block (8 blocks per batch)

_BUILD_CACHE = {}

# Steer the act-table chooser to the single table that holds BOTH ln and
# exp: keep the table list order (act_func_set_id indexes the real
# act_info.json) but hide exp/ln from every OTHER table so the chooser
# cannot alternate between exp_and_others / natural_log (each switch
# costs a 1.3us table reload).
_orig_get_tables = bacc.get_activation_tables


def _tables_force_nl_exp(arch):
    t = _orig_get_tables(arch)
    name = "natural_log_exp_and_others"
    if name not in t:
        return t
    AFT = mybir.ActivationFunctionType
    out = {}
    for k, funcs in t.items():
        if k != name:
            funcs = funcs - {AFT.Exp, AFT.Ln}
        out[k] = funcs
    return out


bacc.get_activation_tables = _tables_force_nl_exp


def build(num_devices=N_CORES, collective=True, dbg=False):
    key = (num_devices, collective, dbg)
    if key in _BUILD_CACHE:
        return _BUILD_CACHE[key]
    nc = bacc.Bacc("TRN2", target_bir_lowering=False, debug=False,
                   num_devices=num_devices)
    xT = nc.dram_tensor("xT", [DIM, NTOK], bf16, kind="ExternalInput").ap()
    wqk = nc.dram_tensor("wqk", [DIM, 128], bf16, kind="ExternalInput").ap()
    wv = nc.dram_tensor("wv", [DIM, D], bf16, kind="ExternalInput").ap()
    w2 = nc.dram_tensor("w2", [DIM, DIM], bf16, kind="ExternalInput").ap()
    o64 = nc.dram_tensor("o64", [D, 1], bf16, kind="ExternalInput").ap()
    # [512 features, 512 tokens]: cols 0:256 batch-0 block, 256:512 batch-1
    outT = nc.dram_tensor("outT", [DIM, DIM], f32, kind="ExternalOutput").ap()
    if dbg:
        d_qn = nc.dram_tensor("d_qn", [D, NTOK], bf16,
                              kind="ExternalOutput").ap()
        d_kraw = nc.dram_tensor("d_kraw", [D, NTOK], bf16,
                                kind="ExternalOutput").ap()
        d_vo = nc.dram_tensor("d_vo", [128, NS1 * JPC, D + 1], bf16,
                              kind="ExternalOutput").ap()
        d_rks = nc.dram_tensor("d_rks", [128, NS1 * JPC], f32,
                               kind="ExternalOutput").ap()
        d_oc = nc.dram_tensor("d_oc", [D, NTOK], bf16,
                              kind="ExternalOutput").ap()
        d_ag = nc.dram_tensor("d_ag", [128, 4, B * BLK], bf16,
                              kind="ExternalOutput").ap()

    xTr = xT.rearrange("(t p) n -> p t n", p=128)
    w2r = w2.rearrange("(t p) n -> p t n", p=128)
    wqkr = wqk.rearrange("(t p) m -> p t m", p=128)
    wvr = wv.rearrange("(t p) m -> p t m", p=128)
    outTr = outT.rearrange("(mt p) n -> p mt n", p=128)

    with tile.TileContext(nc) as tc:
        with (
            tc.tile_pool(name="persist", bufs=1) as pp,
            tc.tile_pool(name="sb", bufs=2) as sb,
            tc.tile_pool(name="ps", bufs=1, space="PSUM") as ps,
            tc.tile_pool(name="dram", bufs=1, space="DRAM") as dram,
            nc.allow_low_precision(reason="bf16 matmul path"),
        ):
            # ---- persistent weights / constants ----
            wqk_sb = pp.tile([128, 4, 128], bf16)
            wv_sb = pp.tile([128, 4, D], bf16)
            w2_sb = pp.tile([128, 4, DIM], bf16)
            o64_sb = pp.tile([D, 1], bf16)
            or_sb = pp.tile([1, D], bf16)      # ones row for PE broadcast
            nc.gpsimd.memset(or_sb[:], 1.0)
            xt_all = pp.tile([128, 4, NTOK], bf16)  # full xT resident

            # startup DMAs: spread across queues; first-chunk pieces first
            nc.sync.dma_start(wqk_sb[:], wqkr[:])
            nc.sync.dma_start(wv_sb[:], wvr[:])
            nc.sync.dma_start(o64_sb[:], o64[:])
            c0 = slice(0, S1C)
            for t in range(4):
                nc.sync.dma_start(xt_all[:, t, c0], xTr[:, t, c0])
            for ci in range(1, 3):
                pc = slice(ci * S1C, (ci + 1) * S1C)
                nc.sync.dma_start(xt_all[:, :, pc], xTr[:, :, pc])
            for ci in range(3, 6):
                pc = slice(ci * S1C, (ci + 1) * S1C)
                nc.scalar.dma_start(xt_all[:, :, pc], xTr[:, :, pc])
            for ci in range(6, 8):
                pc = slice(ci * S1C, (ci + 1) * S1C)
                nc.gpsimd.dma_start(xt_all[:, :, pc], xTr[:, :, pc])
            nc.scalar.dma_start(w2_sb[:], w2r[:])

            # ---- persistent activations ----
            qk_all = pp.tile([128, NTOK], bf16)   # [qT; kT] raw
            qn_sb = pp.tile([D, NTOK], bf16)      # normalized qT
            kraw_sb = pp.tile([D, NTOK], bf16)    # raw kT at partitions 0-63
            vo_sb = pp.tile([128, NS1 * JPC, D + 1], bf16)  # [v | ones]
            nc.gpsimd.memset(vo_sb[:, :, D:D + 1], 1.0)
            lnk_sb = pp.tile([128, NS1 * JPC], f32)  # ln(sum k^2) per j
            rks_sb = pp.tile([128, NS1 * JPC], f32)  # 1/sqrt(sum k^2) per j
            rcq_sb = pp.tile([1, NTOK], bf16)        # 1/sqrt(mean q^2)

            cc_in = [dram.tile([DIM, BLK], bf16, name=f"cc_in{b}")
                     for b in range(B)]
            cc_out = [dram.tile([DIM, BLK], bf16, name=f"cc_out{b}")
                      for b in range(B)]

            def stage1_pieces(ci):
                cols = slice(ci * S1C, (ci + 1) * S1C)
                st = {}

                def p_qk_a():
                    st["qk_ps"] = ps.tile([128, S1C], f32, tag="s1", bufs=2,
                                          name="qk_ps")
                    for t in range(2):
                        nc.tensor.matmul(st["qk_ps"][:], wqk_sb[:, t, :],
                                         xt_all[:, t, cols],
                                         start=(t == 0), stop=False)

                def p_qk_b():
                    for t in range(2, 4):
                        nc.tensor.matmul(st["qk_ps"][:], wqk_sb[:, t, :],
                                         xt_all[:, t, cols],
                                         start=False, stop=(t == 3))
                    nc.vector.tensor_copy(qk_all[:, cols], st["qk_ps"][:])

                def p_vt():
                    vt_ps = ps.tile([D, S1C], f32, tag="s1", bufs=2,
                                    name="vt_ps")
                    for t in range(4):
                        nc.tensor.matmul(vt_ps[:], wv_sb[:, t, :],
                                         xt_all[:, t, cols],
                                         start=(t == 0), stop=(t == 3))
                    vt = sb.tile([D, S1C], bf16, tag="vtsb", bufs=2,
                                 name="vt")
                    st["vt"] = vt
                    nc.vector.tensor_copy(vt[:], vt_ps[:])

                def p_kside():
                    # raw kT remap to partitions 0-63 (Sync queue DMA)
                    nc.sync.dma_start(kraw_sb[:, cols], qk_all[64:128, cols])
                    ksq = sb.tile([D, S1C], bf16, tag="ksq", bufs=2,
                                  name="ksq")
                    nc.vector.tensor_mul(ksq[:], qk_all[64:128, cols],
                                         qk_all[64:128, cols])
                    stk_ps = ps.tile([128, JPC], f32, tag="s1", bufs=2,
                                     name="stk_ps")
                    for jj in range(JPC):
                        js = slice(jj * 128, (jj + 1) * 128)
                        nc.tensor.matmul(stk_ps[:, jj:jj + 1], ksq[:, js],
                                         o64_sb[:], start=True, stop=True)
                    nc.scalar.activation(
                        lnk_sb[:, ci * JPC:(ci + 1) * JPC], stk_ps[:], AF.Ln)

                def p_trans():
                    for jj in range(JPC):
                        jt = ci * JPC + jj
                        js = slice(jj * 128, (jj + 1) * 128)
                        vtr = sb.tile([128, D], bf16, tag="vtr", bufs=4,
                                      name="vtr")
                        nc.sync.dma_start_transpose(vtr[:], st["vt"][:, js])
                        nc.vector.tensor_copy(vo_sb[:, jt, 0:D], vtr[:])

                def p_qside():
                    sq_q = sb.tile([D, S1C], bf16, tag="sqq", bufs=2,
                                   name="sq_q")
                    nc.vector.tensor_mul(sq_q[:], qk_all[0:D, cols],
                                         qk_all[0:D, cols])
                    stq_ps = ps.tile([1, S1C], f32, tag="s1", bufs=2,
                                     name="stq_ps")
                    nc.tensor.matmul(stq_ps[:], o64_sb[:], sq_q[:],
                                     start=True, stop=True)
                    lnq = sb.tile([1, S1C], f32, tag="lnq", bufs=2,
                                  name="lnq")
                    nc.scalar.activation(lnq[:], stq_ps[:], AF.Ln,
                                         scale=1.0 / D)
                    nc.scalar.activation(rcq_sb[:, cols], lnq[:], AF.Exp,
                                         scale=-0.5)
                    rb = sb.tile([D, S1C], bf16, tag="rbsb", bufs=2,
                                 name="rb")
                    nc.gpsimd.partition_broadcast(rb[:], rcq_sb[:, cols])
                    nc.vector.tensor_mul(qn_sb[:, cols], qk_all[0:D, cols],
                                         rb[:])

                return [p_qk_a, p_qk_b, p_vt, p_kside, p_trans, p_qside]

            def rks_batch(h):
                hs = slice(h * JPB, (h + 1) * JPB)
                nc.scalar.activation(rks_sb[:, hs], lnk_sb[:, hs], AF.Exp,
                                     scale=-0.5)

            def phase2_chunk(b, c, fill):
                i0 = b * SEQ + c * P2C
                av_h = [ps.tile([D + 1, HW], f32, tag=f"av{h}", bufs=1,
                                name=f"av{h}")
                        for h in range(2)]

                def sim_mm(jj):
                    j0 = b * SEQ + jj * 128
                    sim_ps = ps.tile([128, 2, HW], f32, tag="sim", bufs=2,
                                     name="sim_ps")
                    for half in range(2):
                        ih = i0 + half * HW
                        nc.tensor.matmul(sim_ps[:, half, :],
                                         kraw_sb[:, j0:j0 + 128],
                                         qn_sb[:, ih:ih + HW],
                                         start=True, stop=True)
                    return sim_ps

                sim_ps = sim_mm(0)
                for jj in range(JPB):
                    gj = b * JPB + jj
                    expT = sb.tile([128, 2, HW], bf16, tag="exp", bufs=3,
                                   name="expT")
                    nc.scalar.activation(expT[:], sim_ps[:], AF.Exp,
                                         scale=rks_sb[:, gj:gj + 1])
                    if jj + 1 < JPB:
                        sim_ps = sim_mm(jj + 1)
                    for half in range(2):
                        nc.tensor.matmul(av_h[half][:],
                                         vo_sb[:, gj, :],
                                         expT[:, half, :],
                                         start=(jj == 0), stop=(jj == JPB - 1))
                    if fill:
                        piece = fill.popleft()
                        piece()
                # copy av out of PSUM (frees the banks for the next chunk)
                avc = sb.tile([D + 1, P2C], f32, tag="avc", bufs=2,
                              name="avc")
                for half in range(2):
                    nc.vector.tensor_copy(avc[:, half * HW:(half + 1) * HW],
                                          av_h[half][:])
                # the rest of the oc chain is DEFERRED: returned as pieces
                # that the caller interleaves into the NEXT chunk, so the
                # PE/DVE never head-of-line block on this chunk's epilogue.
                st = {}

                def p_recip():
                    # Z = av row 64: hop to partition 0 by DMA, approx-recip
                    zrow = sb.tile([1, P2C], f32, tag="zrow", bufs=2,
                                   name="zrow")
                    nc.sync.dma_start(zrow[:], avc[D:D + 1, :])
                    rse = sb.tile([1, P2C], f32, tag="rse", bufs=2,
                                  name="rse")
                    nc.vector.reciprocal_approx_fast(out=rse[:], in_=zrow[:])
                    rsb = sb.tile([1, P2C], bf16, tag="rsb", bufs=2,
                                  name="rsb")
                    nc.vector.tensor_copy(rsb[:], rse[:])
                    st["rsb"] = rsb
                    st["oc"] = sb.tile([D, P2C], bf16, tag="oc", bufs=2,
                                       name="oc")

                def p_oc(half):
                    def run():
                        hs = slice(half * HW, (half + 1) * HW)
                        r2_ps = ps.tile([D, HW], f32, tag="s1", bufs=2,
                                        name="r2_ps")
                        nc.tensor.matmul(r2_ps[:], or_sb[:],
                                         st["rsb"][:, hs],
                                         start=True, stop=True)
                        nc.vector.tensor_tensor(st["oc"][:, hs],
                                                avc[0:D, hs],
                                                r2_ps[:], ALU.mult)
                    return run

                def p_cc():
                    # shard-transpose staging: 4 blocks of 256 tokens
                    oc = st["oc"]
                    for s in range(4):
                        g = c * 4 + s
                        nc.sync.dma_start(cc_in[b][g * D:(g + 1) * D, :],
                                          oc[:, s * BLK:(s + 1) * BLK])
                    if dbg:
                        nc.sync.dma_start(d_oc[:, i0:i0 + P2C], oc[:])

                return [p_recip, p_oc(0), p_oc(1), p_cc]

            def a2a(b):
                if collective:
                    nc.gpsimd.collective_compute(
                        "AllToAll", ALU.bypass,
                        replica_groups=[list(range(num_devices))],
                        ins=[cc_in[b][:]], outs=[cc_out[b][:]])
                else:
                    nc.sync.dma_start(cc_out[b][:], cc_in[b][:])

            def outproj_pieces(b):
                st = {}

                def p_ag():
                    ag = sb.tile([128, 4, BLK], bf16, tag="ag", bufs=2,
                                 name="ag")
                    st["ag"] = ag
                    nc.sync.dma_start(
                        ag[:], cc_out[b].rearrange("(t p) n -> p t n", p=128))
                    if dbg:
                        nc.sync.dma_start(d_ag[:, :, b * BLK:(b + 1) * BLK],
                                          ag[:])
                    st["fo"] = sb.tile([128, 4, BLK], f32, tag="fo", bufs=2,
                                       name="fo")

                def p_mt(mt):
                    def run():
                        fp_ps = ps.tile([128, S1C], f32, tag="s1", bufs=2,
                                        name="fp_ps")
                        for t in range(4):
                            nc.tensor.matmul(
                                fp_ps[:, 0:BLK],
                                w2_sb[:, t, mt * 128:(mt + 1) * 128],
                                st["ag"][:, t, :],
                                start=(t == 0), stop=(t == 3))
                        nc.vector.tensor_copy(st["fo"][:, mt, :],
                                              fp_ps[:, 0:BLK])
                    return run

                def p_out():
                    nc.sync.dma_start(outTr[:, :, b * BLK:(b + 1) * BLK],
                                      st["fo"][:])

                return [p_ag] + [p_mt(mt) for mt in range(4)] + [p_out]

            # ---- schedule ----
            for ci in range(4):
                for p in stage1_pieces(ci):
                    p()
            rks_batch(0)
            fill = collections.deque()
            fill.extend(stage1_pieces(4))
            fill.extend(stage1_pieces(5))
            post = phase2_chunk(0, 0, fill)
            while fill:
                fill.popleft()()
            fill.extend(post)
            fill.extend(stage1_pieces(6))
            fill.extend(stage1_pieces(7))
            fill.append(lambda: rks_batch(1))
            post = phase2_chunk(0, 1, fill)
            while fill:
                fill.popleft()()
            fill.extend(post)
            post = phase2_chunk(1, 0, fill)
            while fill:
                fill.popleft()()
            a2a(0)
            fill.extend(post)
            fill.extend(outproj_pieces(0))
            post = phase2_chunk(1, 1, fill)
            while fill:
                fill.popleft()()
            for p in post:
                p()
            a2a(1)
            for p in outproj_pieces(1):
                p()
            if dbg:
                nc.sync.dma_start(d_qn[:], qn_sb[:])
                nc.sync.dma_start(d_kraw[:], kraw_sb[:])
                nc.sync.dma_start(d_vo[:], vo_sb[:])
                nc.sync.dma_start(d_rks[:], rks_sb[:])
    nc.compile()
    _BUILD_CACHE[key] = nc
    return nc


def make_in_maps(x, Wq, Wkv, Wout):
    xT = np.ascontiguousarray(
        x.reshape(NTOK, DIM).T).astype(ml_dtypes.bfloat16)
    w2 = np.ascontiguousarray(Wout.T).astype(ml_dtypes.bfloat16)
    o64 = np.ones((D, 1), ml_dtypes.bfloat16)
    in_maps = []
    for c in range(N_CORES):
        rows = slice(c * D, (c + 1) * D)
        wqk = np.ascontiguousarray(
            np.concatenate([Wq[rows, :].T, Wkv[rows, :].T],
                           axis=1)).astype(ml_dtypes.bfloat16)
        wv = np.ascontiguousarray(
            Wkv[DIM + c * D:DIM + (c + 1) * D, :].T).astype(ml_dtypes.bfloat16)
        in_maps.append({
            "xT": xT, "wqk": wqk, "wv": wv, "w2": w2, "o64": o64,
        })
    return in_maps


def kernel(x, Wq, Wkv, Wout, _trace=False, _collective=True, _dbg=False):
    nc = build(collective=_collective, dbg=_dbg)
    in_maps = make_in_maps(np.asarray(x), np.asarray(Wq), np.asarray(Wkv),
                           np.asarray(Wout))
    res = bass_utils.run_bass_kernel_spmd(
        nc, in_maps, core_ids=list(range(N_CORES)), trace=_trace)
    full = np.empty((B, SEQ, DIM), np.float32)
    for c in range(N_CORES):
        o = res.results[c]["outT"]  # [512 feat, 512 tok]
        full[0, c * BLK:(c + 1) * BLK, :] = o[:, 0:BLK].T
        full[1, c * BLK:(c + 1) * BLK, :] = o[:, BLK:2 * BLK].T
    if _trace or _dbg:
        return full, res
    return full


# revision 24
# speedup vs baseline: 1.0370x; 1.0370x over previous
"""CosineAttention on 8 TRN2 NeuronCores — v3.

Sharding: head-parallel attention + split AllToAll shard-transpose +
token-parallel out-projection (one head per core, both batches).

Key structure (per core):
  stage 1 (per 512-token chunk): [q;k]T and vT via weight-stationary bf16
    matmuls over resident xT; vT XBAR-transposed (Sync queue) into packed
    [j, 64] vo tiles with a trailing ones column; all sqrt/rsqrt math runs
    as Ln -> Exp(scale) on ACT so ONE activation table serves the whole
    kernel (zero reloads).
  phase 2 (per 1024-token i-chunk): simT = kraw^T qn per j-tile into a
    2-bank PSUM pair, ONE exp instruction per j-tile ([128, 2x512], scale
    AP = 1/|k|), attn@[v|1] accumulates a single PSUM group; softmax
    denominator reciprocal via reciprocal_approx_fast (DVE) + a
    contraction-1 PE matmul broadcast (keeps the Pool queue free for the
    collectives).
  stage-1 work for later chunks is interleaved into phase 2 at j-tile
    granularity so neither ACT nor PE ever drains.
  TWO AllToAlls (one per batch, 256-token blocks): the first overlaps
    batch-1 attention; out-projection of batch 0 overlaps batch-1 tail.
"""

import collections

import numpy as np
import ml_dtypes

import concourse.bass as bass
import concourse.tile as tile
from concourse import bacc
import concourse.mybir as mybir
from concourse import bass_utils

f32 = mybir.dt.float32
bf16 = mybir.dt.bfloat16
AF = mybir.ActivationFunctionType
ALU = mybir.AluOpType

N_CORES = 8
HEADS = 8
D = 64            # head dim
B = 2             # batch
SEQ = 2048        # tokens per batch
DIM = 512         # model dim
NTOK = B * SEQ    # 4096

S1C = 512         # stage-1 token chunk
NS1 = NTOK // S1C          # 8
JPC = S1C // 128           # 4 j-tiles per stage-1 chunk
JPB = SEQ // 128           # 16 j-tiles per batch
P2C = 1024        # phase-2 i-chunk
HW = P2C // 2
BLK = 512         # AllToAll token block (8 global blocks)

_BUILD_CACHE = {}

# Steer the act-table chooser to the single table that holds BOTH ln and
# exp: keep the table list order (act_func_set_id indexes the real
# act_info.json) but hide exp/ln from every OTHER table so the chooser
# cannot alternate between exp_and_others / natural_log (each switch
# costs a 1.3us table reload).
_orig_get_tables = bacc.get_activation_tables


def _tables_force_nl_exp(arch):
    t = _orig_get_tables(arch)
    name = "natural_log_exp_and_others"
    if name not in t:
        return t
    AFT = mybir.ActivationFunctionType
    out = {}
    for k, funcs in t.items():
        if k != name:
            funcs = funcs - {AFT.Exp, AFT.Ln}
        out[k] = funcs
    return out


bacc.get_activation_tables = _tables_force_nl_exp


def build(num_devices=N_CORES, collective=True, dbg=False):
    key = (num_devices, collective, dbg)
    if key in _BUILD_CACHE:
        return _BUILD_CACHE[key]
    nc = bacc.Bacc("TRN2", target_bir_lowering=False, debug=False,
                   num_devices=num_devices)
    xT = nc.dram_tensor("xT", [DIM, NTOK], bf16, kind="ExternalInput").ap()
    wqk = nc.dram_tensor("wqk", [DIM, 128], bf16, kind="ExternalInput").ap()
    wv = nc.dram_tensor("wv", [DIM, D], bf16, kind="ExternalInput").ap()
    w2 = nc.dram_tensor("w2", [DIM, DIM], bf16, kind="ExternalInput").ap()
    o64 = nc.dram_tensor("o64", [D, 1], bf16, kind="ExternalInput").ap()
    # [512 features, 512 tokens]: cols 0:256 batch-0 block, 256:512 batch-1
    outT = nc.dram_tensor("outT", [DIM, DIM], f32, kind="ExternalOutput").ap()
    if dbg:
        d_qn = nc.dram_tensor("d_qn", [D, NTOK], bf16,
                              kind="ExternalOutput").ap()
        d_kraw = nc.dram_tensor("d_kraw", [D, NTOK], bf16,
                                kind="ExternalOutput").ap()
        d_vo = nc.dram_tensor("d_vo", [128, NS1 * JPC, D + 1], bf16,
                              kind="ExternalOutput").ap()
        d_rks = nc.dram_tensor("d_rks", [128, NS1 * JPC], f32,
                               kind="ExternalOutput").ap()
        d_oc = nc.dram_tensor("d_oc", [D, NTOK], bf16,
                              kind="ExternalOutput").ap()
        d_ag = nc.dram_tensor("d_ag", [128, 4, BLK], bf16,
                              kind="ExternalOutput").ap()

    xTr = xT.rearrange("(t p) n -> p t n", p=128)
    w2r = w2.rearrange("(t p) n -> p t n", p=128)
    wqkr = wqk.rearrange("(t p) m -> p t m", p=128)
    wvr = wv.rearrange("(t p) m -> p t m", p=128)
    outTr = outT.rearrange("(mt p) n -> p mt n", p=128)

    with tile.TileContext(nc) as tc:
        with (
            tc.tile_pool(name="persist", bufs=1) as pp,
            tc.tile_pool(name="sb", bufs=2) as sb,
            tc.tile_pool(name="ps", bufs=1, space="PSUM") as ps,
            tc.tile_pool(name="dram", bufs=1, space="DRAM") as dram,
            nc.allow_low_precision(reason="bf16 matmul path"),
        ):
            # ---- persistent weights / constants ----
            wqk_sb = pp.tile([128, 4, 128], bf16)
            wv_sb = pp.tile([128, 4, D], bf16)
            w2_sb = pp.tile([128, 4, DIM], bf16)
            o64_sb = pp.tile([D, 1], bf16)
            or_sb = pp.tile([1, D], bf16)      # ones row for PE broadcast
            nc.gpsimd.memset(or_sb[:], 1.0)
            xt_all = pp.tile([128, 4, NTOK], bf16)  # full xT resident

            # startup DMAs: spread across queues; first-chunk pieces first
            nc.sync.dma_start(wqk_sb[:], wqkr[:])
            nc.sync.dma_start(wv_sb[:], wvr[:])
            nc.sync.dma_start(o64_sb[:], o64[:])
            c0 = slice(0, S1C)
            for t in range(4):
                nc.sync.dma_start(xt_all[:, t, c0], xTr[:, t, c0])
            for ci in range(1, 3):
                pc = slice(ci * S1C, (ci + 1) * S1C)
                nc.sync.dma_start(xt_all[:, :, pc], xTr[:, :, pc])
            for ci in range(3, 6):
                pc = slice(ci * S1C, (ci + 1) * S1C)
                nc.scalar.dma_start(xt_all[:, :, pc], xTr[:, :, pc])
            for ci in range(6, 8):
                pc = slice(ci * S1C, (ci + 1) * S1C)
                nc.gpsimd.dma_start(xt_all[:, :, pc], xTr[:, :, pc])
            nc.scalar.dma_start(w2_sb[:], w2r[:])

            # ---- persistent activations ----
            qk_all = pp.tile([128, NTOK], bf16)   # [qT; kT] raw
            qn_sb = pp.tile([D, NTOK], bf16)      # normalized qT
            kraw_sb = pp.tile([D, NTOK], bf16)    # raw kT at partitions 0-63
            vo_sb = pp.tile([128, NS1 * JPC, D + 1], bf16)  # [v | ones]
            nc.gpsimd.memset(vo_sb[:, :, D:D + 1], 1.0)
            lnk_sb = pp.tile([128, NS1 * JPC], f32)  # ln(sum k^2) per j
            rks_sb = pp.tile([128, NS1 * JPC], f32)  # 1/sqrt(sum k^2) per j
            rcq_sb = pp.tile([1, NTOK], bf16)        # 1/sqrt(mean q^2)

            cc_in = dram.tile([DIM, BLK], bf16, name="cc_in")
            cc_out = dram.tile([DIM, BLK], bf16, name="cc_out")

            def stage1_pieces(ci):
                cols = slice(ci * S1C, (ci + 1) * S1C)
                st = {}

                def p_qk_a():
                    st["qk_ps"] = ps.tile([128, S1C], f32, tag="s1", bufs=2,
                                          name="qk_ps")
                    for t in range(2):
                        nc.tensor.matmul(st["qk_ps"][:], wqk_sb[:, t, :],
                                         xt_all[:, t, cols],
                                         start=(t == 0), stop=False)

                def p_qk_b():
                    for t in range(2, 4):
                        nc.tensor.matmul(st["qk_ps"][:], wqk_sb[:, t, :],
                                         xt_all[:, t, cols],
                                         start=False, stop=(t == 3))
                    nc.vector.tensor_copy(qk_all[:, cols], st["qk_ps"][:])

                def p_vt():
                    vt_ps = ps.tile([D, S1C], f32, tag="s1", bufs=2,
                                    name="vt_ps")
                    for t in range(4):
                        nc.tensor.matmul(vt_ps[:], wv_sb[:, t, :],
                                         xt_all[:, t, cols],
                                         start=(t == 0), stop=(t == 3))
                    vt = sb.tile([D, S1C], bf16, tag="vtsb", bufs=2,
                                 name="vt")
                    st["vt"] = vt
                    nc.vector.tensor_copy(vt[:], vt_ps[:])

                def p_kside():
                    # raw kT remap to partitions 0-63 (Sync queue DMA)
                    nc.sync.dma_start(kraw_sb[:, cols], qk_all[64:128, cols])
                    ksq = sb.tile([D, S1C], bf16, tag="ksq", bufs=2,
                                  name="ksq")
                    nc.vector.tensor_mul(ksq[:], qk_all[64:128, cols],
                                         qk_all[64:128, cols])
                    stk_ps = ps.tile([128, JPC], f32, tag="s1", bufs=2,
                                     name="stk_ps")
                    for jj in range(JPC):
                        js = slice(jj * 128, (jj + 1) * 128)
                        nc.tensor.matmul(stk_ps[:, jj:jj + 1], ksq[:, js],
                                         o64_sb[:], start=True, stop=True)
                    nc.scalar.activation(
                        lnk_sb[:, ci * JPC:(ci + 1) * JPC], stk_ps[:], AF.Ln)

                def p_trans():
                    for jj in range(JPC):
                        jt = ci * JPC + jj
                        js = slice(jj * 128, (jj + 1) * 128)
                        vtr = sb.tile([128, D], bf16, tag="vtr", bufs=4,
                                      name="vtr")
                        nc.sync.dma_start_transpose(vtr[:], st["vt"][:, js])
                        nc.vector.tensor_copy(vo_sb[:, jt, 0:D], vtr[:])

                def p_qside():
                    sq_q = sb.tile([D, S1C], bf16, tag="sqq", bufs=2,
                                   name="sq_q")
                    nc.vector.tensor_mul(sq_q[:], qk_all[0:D, cols],
                                         qk_all[0:D, cols])
                    stq_ps = ps.tile([1, S1C], f32, tag="s1", bufs=2,
                                     name="stq_ps")
                    nc.tensor.matmul(stq_ps[:], o64_sb[:], sq_q[:],
                                     start=True, stop=True)
                    lnq = sb.tile([1, S1C], f32, tag="lnq", bufs=2,
                                  name="lnq")
                    nc.scalar.activation(lnq[:], stq_ps[:], AF.Ln,
                                         scale=1.0 / D)
                    nc.scalar.activation(rcq_sb[:, cols], lnq[:], AF.Exp,
                                         scale=-0.5)
                    rb = sb.tile([D, S1C], bf16, tag="rbsb", bufs=2,
                                 name="rb")
                    nc.gpsimd.partition_broadcast(rb[:], rcq_sb[:, cols])
                    nc.vector.tensor_mul(qn_sb[:, cols], qk_all[0:D, cols],
                                         rb[:])

                return [p_qk_a, p_qk_b, p_vt, p_kside, p_trans, p_qside]

            def rks_batch(h):
                hs = slice(h * JPB, (h + 1) * JPB)
                nc.scalar.activation(rks_sb[:, hs], lnk_sb[:, hs], AF.Exp,
                                     scale=-0.5)

            def phase2_chunk(b, c, fill):
                i0 = b * SEQ + c * P2C
                av_h = [ps.tile([D + 1, HW], f32, tag=f"av{h}", bufs=1,
                                name=f"av{h}")
                        for h in range(2)]

                def sim_mm(jj):
                    j0 = b * SEQ + jj * 128
                    sim_ps = ps.tile([128, 2, HW], f32, tag="sim", bufs=2,
                                     name="sim_ps")
                    for half in range(2):
                        ih = i0 + half * HW
                        nc.tensor.matmul(sim_ps[:, half, :],
                                         kraw_sb[:, j0:j0 + 128],
                                         qn_sb[:, ih:ih + HW],
                                         start=True, stop=True)
                    return sim_ps

                sim_ps = sim_mm(0)
                for jj in range(JPB):
                    gj = b * JPB + jj
                    expT = sb.tile([128, 2, HW], bf16, tag="exp", bufs=3,
                                   name="expT")
                    nc.scalar.activation(expT[:], sim_ps[:], AF.Exp,
                                         scale=rks_sb[:, gj:gj + 1])
                    if jj + 1 < JPB:
                        sim_ps = sim_mm(jj + 1)
                    for half in range(2):
                        nc.tensor.matmul(av_h[half][:],
                                         vo_sb[:, gj, :],
                                         expT[:, half, :],
                                         start=(jj == 0), stop=(jj == JPB - 1))
                    if fill:
                        piece = fill.popleft()
                        piece()
                # copy av out of PSUM (frees the banks for the next chunk)
                avc = sb.tile([D + 1, P2C], f32, tag="avc", bufs=2,
                              name="avc")
                for half in range(2):
                    nc.vector.tensor_copy(avc[:, half * HW:(half + 1) * HW],
                                          av_h[half][:])
                # Z = av row 64: hop to partition 0 by DMA, approx-recip,
                # broadcast on Pool, multiply on DVE
                zrow = sb.tile([1, P2C], f32, tag="zrow", bufs=2,
                               name="zrow")
                nc.sync.dma_start(zrow[:], avc[D:D + 1, :])
                rse = sb.tile([1, P2C], f32, tag="rse", bufs=2, name="rse")
                nc.vector.reciprocal_approx_fast(out=rse[:], in_=zrow[:])
                r2 = sb.tile([D, P2C], f32, tag="r2sb", bufs=2, name="r2")
                nc.gpsimd.partition_broadcast(r2[:], rse[:])
                oc = sb.tile([D, P2C], bf16, tag="oc", bufs=2, name="oc")
                nc.vector.tensor_tensor(oc[:], avc[0:D, :], r2[:], ALU.mult)
                # shard-transpose staging: 2 half-blocks of 512 tokens
                for s in range(2):
                    g = (b * SEQ + c * P2C) // BLK + s
                    nc.sync.dma_start(cc_in[g * D:(g + 1) * D, :],
                                      oc[:, s * 512:(s + 1) * 512])
                if dbg:
                    nc.sync.dma_start(d_oc[:, i0:i0 + P2C], oc[:])
                return []

            def a2a():
                if collective:
                    nc.gpsimd.collective_compute(
                        "AllToAll", ALU.bypass,
                        replica_groups=[list(range(num_devices))],
                        ins=[cc_in[:]], outs=[cc_out[:]])
                else:
                    nc.sync.dma_start(cc_out[:], cc_in[:])

            def outproj():
                ag = sb.tile([128, 4, BLK], bf16, tag="ag", bufs=2,
                             name="ag")
                nc.sync.dma_start(
                    ag[:], cc_out.rearrange("(t p) n -> p t n", p=128))
                if dbg:
                    nc.sync.dma_start(d_ag[:], ag[:])
                fo = sb.tile([128, 4, BLK], f32, tag="fo", bufs=2,
                             name="fo")
                for mt in range(4):
                    fp_ps = ps.tile([128, S1C], f32, tag="s1", bufs=2,
                                    name="fp_ps")
                    for t in range(4):
                        nc.tensor.matmul(
                            fp_ps[:],
                            w2_sb[:, t, mt * 128:(mt + 1) * 128],
                            ag[:, t, :],
                            start=(t == 0), stop=(t == 3))
                    nc.vector.tensor_copy(fo[:, mt, :], fp_ps[:])
                nc.sync.dma_start(outTr[:], fo[:])

            # ---- schedule ----
            for ci in range(4):
                for p in stage1_pieces(ci):
                    p()
            rks_batch(0)
            empty = collections.deque()
            phase2_chunk(0, 0, empty)
            for p in stage1_pieces(4):
                p()
            for p in stage1_pieces(5):
                p()
            phase2_chunk(0, 1, empty)
            for p in stage1_pieces(6):
                p()
            for p in stage1_pieces(7):
                p()
            rks_batch(1)
            phase2_chunk(1, 0, empty)
            phase2_chunk(1, 1, empty)
            a2a()
            outproj()
            if dbg:
                nc.sync.dma_start(d_qn[:], qn_sb[:])
                nc.sync.dma_start(d_kraw[:], kraw_sb[:])
                nc.sync.dma_start(d_vo[:], vo_sb[:])
                nc.sync.dma_start(d_rks[:], rks_sb[:])
    nc.compile()
    _BUILD_CACHE[key] = nc
    return nc


def make_in_maps(x, Wq, Wkv, Wout):
    xT = np.ascontiguousarray(
        x.reshape(NTOK, DIM).T).astype(ml_dtypes.bfloat16)
    w2 = np.ascontiguousarray(Wout.T).astype(ml_dtypes.bfloat16)
    o64 = np.ones((D, 1), ml_dtypes.bfloat16)
    in_maps = []
    for c in range(N_CORES):
        rows = slice(c * D, (c + 1) * D)
        wqk = np.ascontiguousarray(
            np.concatenate([Wq[rows, :].T, Wkv[rows, :].T],
                           axis=1)).astype(ml_dtypes.bfloat16)
        wv = np.ascontiguousarray(
            Wkv[DIM + c * D:DIM + (c + 1) * D, :].T).astype(ml_dtypes.bfloat16)
        in_maps.append({
            "xT": xT, "wqk": wqk, "wv": wv, "w2": w2, "o64": o64,
        })
    return in_maps


def kernel(x, Wq, Wkv, Wout, _trace=False, _collective=True, _dbg=False):
    nc = build(collective=_collective, dbg=_dbg)
    in_maps = make_in_maps(np.asarray(x), np.asarray(Wq), np.asarray(Wkv),
                           np.asarray(Wout))
    res = bass_utils.run_bass_kernel_spmd(
        nc, in_maps, core_ids=list(range(N_CORES)), trace=_trace)
    full = np.empty((B, SEQ, DIM), np.float32)
    flat = full.reshape(NTOK, DIM)
    for c in range(N_CORES):
        o = res.results[c]["outT"]  # [512 feat, 512 global tok block c]
        flat[c * BLK:(c + 1) * BLK, :] = o.T
    if _trace or _dbg:
        return full, res
    return full


# revision 25
# speedup vs baseline: 1.0925x; 1.0535x over previous
"""CosineAttention on 8 TRN2 NeuronCores — v3.

Sharding: head-parallel attention + split AllToAll shard-transpose +
token-parallel out-projection (one head per core, both batches).

Key structure (per core):
  stage 1 (per 512-token chunk): [q;k]T and vT via weight-stationary bf16
    matmuls over resident xT; vT XBAR-transposed (Sync queue) into packed
    [j, 64] vo tiles with a trailing ones column; all sqrt/rsqrt math runs
    as Ln -> Exp(scale) on ACT so ONE activation table serves the whole
    kernel (zero reloads).
  phase 2 (per 1024-token i-chunk): simT = kraw^T qn per j-tile into a
    2-bank PSUM pair, ONE exp instruction per j-tile ([128, 2x512], scale
    AP = 1/|k|), attn@[v|1] accumulates a single PSUM group; softmax
    denominator reciprocal via reciprocal_approx_fast (DVE) + a
    contraction-1 PE matmul broadcast (keeps the Pool queue free for the
    collectives).
  stage-1 work for later chunks is interleaved into phase 2 at j-tile
    granularity so neither ACT nor PE ever drains.
  TWO AllToAlls (one per batch, 256-token blocks): the first overlaps
    batch-1 attention; out-projection of batch 0 overlaps batch-1 tail.
"""

import collections

import numpy as np
import ml_dtypes

import concourse.bass as bass
import concourse.tile as tile
from concourse import bacc
import concourse.mybir as mybir
from concourse import bass_utils

f32 = mybir.dt.float32
bf16 = mybir.dt.bfloat16
AF = mybir.ActivationFunctionType
ALU = mybir.AluOpType

N_CORES = 8
HEADS = 8
D = 64            # head dim
B = 2             # batch
SEQ = 2048        # tokens per batch
DIM = 512         # model dim
NTOK = B * SEQ    # 4096

S1C = 512         # stage-1 token chunk
NS1 = NTOK // S1C          # 8
JPC = S1C // 128           # 4 j-tiles per stage-1 chunk
JPB = SEQ // 128           # 16 j-tiles per batch
P2C = 1024        # phase-2 i-chunk
HW = P2C // 2
BLK = 512         # AllToAll token block (8 global blocks)

_BUILD_CACHE = {}

# Steer the act-table chooser to the single table that holds BOTH ln and
# exp: keep the table list order (act_func_set_id indexes the real
# act_info.json) but hide exp/ln from every OTHER table so the chooser
# cannot alternate between exp_and_others / natural_log (each switch
# costs a 1.3us table reload).
_orig_get_tables = bacc.get_activation_tables


def _tables_force_nl_exp(arch):
    t = _orig_get_tables(arch)
    name = "natural_log_exp_and_others"
    if name not in t:
        return t
    AFT = mybir.ActivationFunctionType
    out = {}
    for k, funcs in t.items():
        if k != name:
            funcs = funcs - {AFT.Exp, AFT.Ln}
        out[k] = funcs
    return out


bacc.get_activation_tables = _tables_force_nl_exp


def build(num_devices=N_CORES, collective=True, dbg=False):
    key = (num_devices, collective, dbg)
    if key in _BUILD_CACHE:
        return _BUILD_CACHE[key]
    nc = bacc.Bacc("TRN2", target_bir_lowering=False, debug=False,
                   num_devices=num_devices)
    xT = nc.dram_tensor("xT", [DIM, NTOK], bf16, kind="ExternalInput").ap()
    wqk = nc.dram_tensor("wqk", [DIM, 128], bf16, kind="ExternalInput").ap()
    wv = nc.dram_tensor("wv", [DIM, D], bf16, kind="ExternalInput").ap()
    w2 = nc.dram_tensor("w2", [DIM, DIM], bf16, kind="ExternalInput").ap()
    o64 = nc.dram_tensor("o64", [D, 1], bf16, kind="ExternalInput").ap()
    # [512 features, 512 tokens]: cols 0:256 batch-0 block, 256:512 batch-1
    outT = nc.dram_tensor("outT", [DIM, DIM], f32, kind="ExternalOutput").ap()
    if dbg:
        d_qn = nc.dram_tensor("d_qn", [D, NTOK], bf16,
                              kind="ExternalOutput").ap()
        d_kraw = nc.dram_tensor("d_kraw", [D, NTOK], bf16,
                                kind="ExternalOutput").ap()
        d_vo = nc.dram_tensor("d_vo", [128, NS1 * JPC, D + 1], bf16,
                              kind="ExternalOutput").ap()
        d_rks = nc.dram_tensor("d_rks", [128, NS1 * JPC], f32,
                               kind="ExternalOutput").ap()
        d_oc = nc.dram_tensor("d_oc", [D, NTOK], bf16,
                              kind="ExternalOutput").ap()
        d_ag = nc.dram_tensor("d_ag", [128, 4, BLK], bf16,
                              kind="ExternalOutput").ap()

    xTr = xT.rearrange("(t p) n -> p t n", p=128)
    w2r = w2.rearrange("(t p) n -> p t n", p=128)
    wqkr = wqk.rearrange("(t p) m -> p t m", p=128)
    wvr = wv.rearrange("(t p) m -> p t m", p=128)
    outTr = outT.rearrange("(mt p) n -> p mt n", p=128)

    with tile.TileContext(nc) as tc:
        with (
            tc.tile_pool(name="persist", bufs=1) as pp,
            tc.tile_pool(name="sb", bufs=2) as sb,
            tc.tile_pool(name="ps", bufs=1, space="PSUM") as ps,
            tc.tile_pool(name="dram", bufs=1, space="DRAM") as dram,
            nc.allow_low_precision(reason="bf16 matmul path"),
        ):
            # ---- persistent weights / constants ----
            wqk_sb = pp.tile([128, 4, 128], bf16)
            wv_sb = pp.tile([128, 4, D], bf16)
            w2_sb = pp.tile([128, 4, DIM], bf16)
            o64_sb = pp.tile([D, 1], bf16)
            or_sb = pp.tile([1, D], bf16)      # ones row for PE broadcast
            nc.gpsimd.memset(or_sb[:], 1.0)
            xt_all = pp.tile([128, 4, NTOK], bf16)  # full xT resident

            # startup DMAs: spread across queues; first-chunk pieces first
            nc.sync.dma_start(wqk_sb[:], wqkr[:])
            nc.sync.dma_start(wv_sb[:], wvr[:])
            nc.sync.dma_start(o64_sb[:], o64[:])
            c0 = slice(0, S1C)
            for t in range(4):
                nc.sync.dma_start(xt_all[:, t, c0], xTr[:, t, c0])
            for ci in range(1, 3):
                pc = slice(ci * S1C, (ci + 1) * S1C)
                nc.sync.dma_start(xt_all[:, :, pc], xTr[:, :, pc])
            for ci in range(3, 6):
                pc = slice(ci * S1C, (ci + 1) * S1C)
                nc.scalar.dma_start(xt_all[:, :, pc], xTr[:, :, pc])
            for ci in range(6, 8):
                pc = slice(ci * S1C, (ci + 1) * S1C)
                nc.gpsimd.dma_start(xt_all[:, :, pc], xTr[:, :, pc])
            nc.scalar.dma_start(w2_sb[:], w2r[:])

            # ---- persistent activations ----
            qk_all = pp.tile([128, NTOK], bf16)   # [qT; kT] raw
            qn_sb = pp.tile([D, NTOK], bf16)      # normalized qT
            kraw_sb = pp.tile([D, NTOK], bf16)    # raw kT at partitions 0-63
            vo_sb = pp.tile([128, NS1 * JPC, D + 1], bf16)  # [v | ones]
            nc.gpsimd.memset(vo_sb[:, :, D:D + 1], 1.0)
            lnk_sb = pp.tile([128, NS1 * JPC], f32)  # ln(sum k^2) per j
            rks_sb = pp.tile([128, NS1 * JPC], f32)  # 1/sqrt(sum k^2) per j
            rcq_sb = pp.tile([1, NTOK], bf16)        # 1/sqrt(mean q^2)

            cc_in = dram.tile([DIM, BLK], bf16, name="cc_in")
            cc_out = dram.tile([DIM, BLK], bf16, name="cc_out")

            def stage1_pieces(ci):
                cols = slice(ci * S1C, (ci + 1) * S1C)
                st = {}

                def p_qk_a():
                    st["qk_ps"] = ps.tile([128, S1C], f32, tag="s1", bufs=2,
                                          name="qk_ps")
                    for t in range(2):
                        nc.tensor.matmul(st["qk_ps"][:], wqk_sb[:, t, :],
                                         xt_all[:, t, cols],
                                         start=(t == 0), stop=False)

                def p_qk_b():
                    for t in range(2, 4):
                        nc.tensor.matmul(st["qk_ps"][:], wqk_sb[:, t, :],
                                         xt_all[:, t, cols],
                                         start=False, stop=(t == 3))
                    nc.vector.tensor_copy(qk_all[:, cols], st["qk_ps"][:])

                def p_vt():
                    vt_ps = ps.tile([D, S1C], f32, tag="s1", bufs=2,
                                    name="vt_ps")
                    for t in range(4):
                        nc.tensor.matmul(vt_ps[:], wv_sb[:, t, :],
                                         xt_all[:, t, cols],
                                         start=(t == 0), stop=(t == 3))
                    vt = sb.tile([D, S1C], bf16, tag="vtsb", bufs=2,
                                 name="vt")
                    st["vt"] = vt
                    nc.vector.tensor_copy(vt[:], vt_ps[:])

                def p_kside():
                    # raw kT remap to partitions 0-63 (Sync queue DMA)
                    nc.sync.dma_start(kraw_sb[:, cols], qk_all[64:128, cols])
                    ksq = sb.tile([D, S1C], bf16, tag="ksq", bufs=2,
                                  name="ksq")
                    nc.vector.tensor_mul(ksq[:], qk_all[64:128, cols],
                                         qk_all[64:128, cols])
                    stk_ps = ps.tile([128, JPC], f32, tag="s1", bufs=2,
                                     name="stk_ps")
                    for jj in range(JPC):
                        js = slice(jj * 128, (jj + 1) * 128)
                        nc.tensor.matmul(stk_ps[:, jj:jj + 1], ksq[:, js],
                                         o64_sb[:], start=True, stop=True)
                    nc.scalar.activation(
                        lnk_sb[:, ci * JPC:(ci + 1) * JPC], stk_ps[:], AF.Ln)

                def p_trans():
                    for jj in range(JPC):
                        jt = ci * JPC + jj
                        js = slice(jj * 128, (jj + 1) * 128)
                        vtr = sb.tile([128, D], bf16, tag="vtr", bufs=4,
                                      name="vtr")
                        nc.sync.dma_start_transpose(vtr[:], st["vt"][:, js])
                        nc.vector.tensor_copy(vo_sb[:, jt, 0:D], vtr[:])

                def p_qside():
                    sq_q = sb.tile([D, S1C], bf16, tag="sqq", bufs=2,
                                   name="sq_q")
                    nc.vector.tensor_mul(sq_q[:], qk_all[0:D, cols],
                                         qk_all[0:D, cols])
                    stq_ps = ps.tile([1, S1C], f32, tag="s1", bufs=2,
                                     name="stq_ps")
                    nc.tensor.matmul(stq_ps[:], o64_sb[:], sq_q[:],
                                     start=True, stop=True)
                    lnq = sb.tile([1, S1C], f32, tag="lnq", bufs=2,
                                  name="lnq")
                    nc.scalar.activation(lnq[:], stq_ps[:], AF.Ln,
                                         scale=1.0 / D)
                    nc.scalar.activation(rcq_sb[:, cols], lnq[:], AF.Exp,
                                         scale=-0.5)
                    rb = sb.tile([D, S1C], bf16, tag="rbsb", bufs=2,
                                 name="rb")
                    nc.gpsimd.partition_broadcast(rb[:], rcq_sb[:, cols])
                    nc.vector.tensor_mul(qn_sb[:, cols], qk_all[0:D, cols],
                                         rb[:])

                return [p_qk_a, p_qk_b, p_vt, p_kside, p_trans, p_qside]

            def rks_batch(h):
                hs = slice(h * JPB, (h + 1) * JPB)
                nc.scalar.activation(rks_sb[:, hs], lnk_sb[:, hs], AF.Exp,
                                     scale=-0.5)

            def phase2_chunk(b, c, fill):
                i0 = b * SEQ + c * P2C
                av_h = [ps.tile([D + 1, HW], f32, tag=f"av{h}", bufs=1,
                                name=f"av{h}")
                        for h in range(2)]

                def sim_mm(jj):
                    j0 = b * SEQ + jj * 128
                    sim_ps = ps.tile([128, 2, HW], f32, tag="sim", bufs=2,
                                     name="sim_ps")
                    for half in range(2):
                        ih = i0 + half * HW
                        nc.tensor.matmul(sim_ps[:, half, :],
                                         kraw_sb[:, j0:j0 + 128],
                                         qn_sb[:, ih:ih + HW],
                                         start=True, stop=True)
                    return sim_ps

                sim_ps = sim_mm(0)
                for jj in range(JPB):
                    gj = b * JPB + jj
                    expT = sb.tile([128, 2, HW], bf16, tag="exp", bufs=3,
                                   name="expT")
                    nc.scalar.activation(expT[:], sim_ps[:], AF.Exp,
                                         scale=rks_sb[:, gj:gj + 1])
                    if jj + 1 < JPB:
                        sim_ps = sim_mm(jj + 1)
                    for half in range(2):
                        nc.tensor.matmul(av_h[half][:],
                                         vo_sb[:, gj, :],
                                         expT[:, half, :],
                                         start=(jj == 0), stop=(jj == JPB - 1))
                    if jj in (5, 11):
                        for _ in range(3):
                            if fill:
                                fill.popleft()()
                # copy av out of PSUM (frees the banks for the next chunk)
                avc = sb.tile([D + 1, P2C], f32, tag="avc", bufs=2,
                              name="avc")
                for half in range(2):
                    nc.vector.tensor_copy(avc[:, half * HW:(half + 1) * HW],
                                          av_h[half][:])
                # Z = av row 64: hop to partition 0 by DMA, approx-recip,
                # broadcast on Pool, multiply on DVE
                zrow = sb.tile([1, P2C], f32, tag="zrow", bufs=2,
                               name="zrow")
                nc.sync.dma_start(zrow[:], avc[D:D + 1, :])
                rse = sb.tile([1, P2C], f32, tag="rse", bufs=2, name="rse")
                nc.vector.reciprocal_approx_fast(out=rse[:], in_=zrow[:])
                r2 = sb.tile([D, P2C], f32, tag="r2sb", bufs=2, name="r2")
                nc.gpsimd.partition_broadcast(r2[:], rse[:])
                oc = sb.tile([D, P2C], bf16, tag="oc", bufs=2, name="oc")
                nc.vector.tensor_tensor(oc[:], avc[0:D, :], r2[:], ALU.mult)
                # shard-transpose staging: 2 half-blocks of 512 tokens
                for s in range(2):
                    g = (b * SEQ + c * P2C) // BLK + s
                    nc.sync.dma_start(cc_in[g * D:(g + 1) * D, :],
                                      oc[:, s * 512:(s + 1) * 512])
                if dbg:
                    nc.sync.dma_start(d_oc[:, i0:i0 + P2C], oc[:])
                return []

            def a2a():
                if collective:
                    nc.gpsimd.collective_compute(
                        "AllToAll", ALU.bypass,
                        replica_groups=[list(range(num_devices))],
                        ins=[cc_in[:]], outs=[cc_out[:]])
                else:
                    nc.sync.dma_start(cc_out[:], cc_in[:])

            def outproj():
                ag = sb.tile([128, 4, BLK], bf16, tag="ag", bufs=2,
                             name="ag")
                nc.sync.dma_start(
                    ag[:], cc_out.rearrange("(t p) n -> p t n", p=128))
                if dbg:
                    nc.sync.dma_start(d_ag[:], ag[:])
                fo = sb.tile([128, 4, BLK], f32, tag="fo", bufs=2,
                             name="fo")
                for mt in range(4):
                    fp_ps = ps.tile([128, S1C], f32, tag="s1", bufs=2,
                                    name="fp_ps")
                    for t in range(4):
                        nc.tensor.matmul(
                            fp_ps[:],
                            w2_sb[:, t, mt * 128:(mt + 1) * 128],
                            ag[:, t, :],
                            start=(t == 0), stop=(t == 3))
                    nc.vector.tensor_copy(fo[:, mt, :], fp_ps[:])
                nc.sync.dma_start(outTr[:], fo[:])

            # ---- schedule ----
            for ci in range(4):
                for p in stage1_pieces(ci):
                    p()
            rks_batch(0)
            fill = collections.deque()
            fill.extend(stage1_pieces(4))
            fill.extend(stage1_pieces(5))
            phase2_chunk(0, 0, fill)
            while fill:
                fill.popleft()()
            fill.extend(stage1_pieces(6))
            fill.extend(stage1_pieces(7))
            fill.append(lambda: rks_batch(1))
            phase2_chunk(0, 1, fill)
            while fill:
                fill.popleft()()
            phase2_chunk(1, 0, fill)
            phase2_chunk(1, 1, fill)
            a2a()
            outproj()
            if dbg:
                nc.sync.dma_start(d_qn[:], qn_sb[:])
                nc.sync.dma_start(d_kraw[:], kraw_sb[:])
                nc.sync.dma_start(d_vo[:], vo_sb[:])
                nc.sync.dma_start(d_rks[:], rks_sb[:])
    nc.compile()
    _BUILD_CACHE[key] = nc
    return nc


def make_in_maps(x, Wq, Wkv, Wout):
    xT = np.ascontiguousarray(
        x.reshape(NTOK, DIM).T).astype(ml_dtypes.bfloat16)
    w2 = np.ascontiguousarray(Wout.T).astype(ml_dtypes.bfloat16)
    o64 = np.ones((D, 1), ml_dtypes.bfloat16)
    in_maps = []
    for c in range(N_CORES):
        rows = slice(c * D, (c + 1) * D)
        wqk = np.ascontiguousarray(
            np.concatenate([Wq[rows, :].T, Wkv[rows, :].T],
                           axis=1)).astype(ml_dtypes.bfloat16)
        wv = np.ascontiguousarray(
            Wkv[DIM + c * D:DIM + (c + 1) * D, :].T).astype(ml_dtypes.bfloat16)
        in_maps.append({
            "xT": xT, "wqk": wqk, "wv": wv, "w2": w2, "o64": o64,
        })
    return in_maps


def kernel(x, Wq, Wkv, Wout, _trace=False, _collective=True, _dbg=False):
    nc = build(collective=_collective, dbg=_dbg)
    in_maps = make_in_maps(np.asarray(x), np.asarray(Wq), np.asarray(Wkv),
                           np.asarray(Wout))
    res = bass_utils.run_bass_kernel_spmd(
        nc, in_maps, core_ids=list(range(N_CORES)), trace=_trace)
    full = np.empty((B, SEQ, DIM), np.float32)
    flat = full.reshape(NTOK, DIM)
    for c in range(N_CORES):
        o = res.results[c]["outT"]  # [512 feat, 512 global tok block c]
        flat[c * BLK:(c + 1) * BLK, :] = o.T
    if _trace or _dbg:
        return full, res
    return full
